# revision 42
# baseline (speedup 1.0000x reference)
"""CapsNet Trainium2 kernel — 8-core data-parallel Bass/Tile, bf16 v2.

Per-core (B_loc=32):
  conv1 (9x9,s1): host-side im2col P1[82,12800] (row 81 = ones for bias),
     w1e[82,256] (row 81 = bias) -> 50 matmuls N=512 -> relu -> h bf16
  primary caps (9x9,s2,256ch): host-transposed pwT[ic,(loc,drdc)] bf16,
     162 accumulated matmuls x 4 batch-chunks per half -> psB
     ACT extracts capsP (+bias) and capsP^2 in one pass each
  squash over j, capsT via PE transposes
  routing (3 iters), U never materialized:
     S = capsS @ (W0*C); V = squash_cls(S)
     X = capsT.T @ V; Y = X*W0; k-sum via DVE tree-adds;
     m-sum + transpose fused into per-w mask matmuls -> bupd[10,1152]
     AllReduce -> b += bupd/256 -> C = softmax(b) (PE-broadcast to partitions)
"""
import sys

for _p in ("/opt/trn_rl_repo",):
    if _p not in sys.path:
        sys.path.insert(0, _p)

import numpy as np
import ml_dtypes

import concourse.bass as bass
import concourse.tile as tile
from concourse import bacc, mybir
from concourse.bass_utils import run_bass_kernel_spmd
from concourse.masks import make_identity

F32 = mybir.dt.float32
BF16 = mybir.dt.bfloat16
AF = mybir.ActivationFunctionType
ALU = mybir.AluOpType
AX = mybir.AxisListType

NC_CORES = 8
BL = 32            # batch per core
R_ITERS = 3
NPBF = ml_dtypes.bfloat16


def build_kernel_body(tc, io, collectives=True):
    nc = tc.nc
    p1_t = io["p1"]      # [82, 12800] bf16   im2col(x) + ones row
    w1_t = io["w1"]      # [82, 256]  bf16    conv1 wT + bias row
    pw_t = io["pw"]      # [256, 20736] bf16  [ic, (loc, drdc)]
    pb_t = io["pb"]      # [256, 1]   f32
    w0_t = io["w0"]      # [256, 5760] bf16   [(m,oc), (w, k, l)]
    out_t = io["out"]    # [32, 160]  f32

    def pool(name, bufs=1, space=None):
        kw = {"space": space} if space else {}
        return tc.tile_pool(name=name, bufs=bufs, **kw)

    with pool("const") as constp, pool("caps") as capsp, \
         pool("arwarm", space="DRAM") as arwp:
        # ---------------- early DMAs (big, overlap with conv1) ----------------
        pw_pool = pool("pw")
        pwp = pw_pool.__enter__()
        pwT = [pwp.tile([128, 20736], BF16, name=f"pwT{i}") for i in range(2)]
        W0 = [capsp.tile([128, 5760], BF16, name=f"W0_{h}") for h in range(2)]

        # dummy AllReduce to absorb collective setup latency (runs under conv1)
        if collectives:
            zwarm = constp.tile([10, 16], F32)
            nc.gpsimd.memset(zwarm[:], 0.0)
            arw_in = arwp.tile([10, 16], F32, name="arw_in")
            arw_out = arwp.tile([10, 16], F32, addr_space="Shared", name="arw_out")
            nc.scalar.dma_start(arw_in[:], zwarm[:])
            nc.gpsimd.collective_compute(
                "AllReduce", ALU.add,
                replica_groups=[list(range(NC_CORES))],
                ins=[arw_in.opt()], outs=[arw_out.opt()])

        # ---------------- constants ----------------
        identF = constp.tile([128, 128], F32)
        make_identity(nc, identF[:])
        identB = constp.tile([128, 128], BF16)
        nc.vector.tensor_copy(identB[:], identF[:])

        # maskL[half]: [128,8] f32, 1 at (p, half*4 + p//32)
        maskL = []
        for half in range(2):
            m = constp.tile([128, 8], F32, name=f"maskL{half}")
            nc.gpsimd.memset(m[:], 0.0)
            for l4 in range(4):
                nc.gpsimd.memset(m[l4 * 32:(l4 + 1) * 32, half * 4 + l4: half * 4 + l4 + 1], 1.0)
            maskL.append(m)

        # maskOC: [128,32] 1 at (p, p%32); bf16 copy for bupd matmuls
        mOCa = constp.tile([128, 32], F32)
        mOCb = constp.tile([128, 32], F32)
        maskOCf = constp.tile([128, 32], F32)
        maskOC = constp.tile([128, 32], BF16)
        nc.vector.tensor_add(mOCa[:], identF[:, 0:32], identF[:, 32:64])
        nc.vector.tensor_add(mOCb[:], identF[:, 64:96], identF[:, 96:128])
        nc.vector.tensor_add(maskOCf[:], mOCa[:], mOCb[:])
        nc.vector.tensor_copy(maskOC[:], maskOCf[:])

        pb_sb = constp.tile([128, 2], F32)   # pri bias, col=half
        for blk in range(2):
            nc.sync.dma_start(pb_sb[:, blk:blk + 1], pb_t[blk * 128:(blk + 1) * 128, 0:1])

        capsP = [capsp.tile([128, 1152], F32, name=f"capsP{h}") for h in range(2)]
        capsS = [capsp.tile([128, 1152], BF16, name=f"capsS{h}") for h in range(2)]
        rs = capsp.tile([128, 64], F32)  # per-(l4,oc) sums of squares, col half*32+b

        # ================= phase 1: conv1 =================
        with pool("hbuf") as hp:
            h_sb = [hp.tile([128, 12800], BF16, name=f"h{b}") for b in range(2)]
            ph1_scope = nc.named_scope("ph1_conv1")
            ph1_scope.__enter__()
            with pool("ph1") as ph1, pool("pp1", bufs=8, space="PSUM") as pp1:
                # PE warm-up: dummy matmuls so HAM un-throttles before conv1
                warm = pp1.tile([128, 128], F32, tag="warm", bufs=1)
                for _ in range(24):
                    nc.tensor.matmul(warm[:], identF[:], identF[:, 0:128],
                                     start=True, stop=True)
                w1sb = ph1.tile([82, 256], BF16)
                nc.sync.dma_start(w1sb[:], w1_t[:, :])
                P1 = ph1.tile([82, 12800], BF16)
                for q in range(4):
                    nc.sync.dma_start(P1[:, q * 3200:(q + 1) * 3200],
                                      p1_t[:, q * 3200:(q + 1) * 3200])
                # Big weight loads must not steal HBM bandwidth from the
                # conv1-critical P1: a tiny write into each target tile that
                # reads the LAST P1 chunk creates a WAW dep, so the pwT DMAs
                # start only after P1 has fully landed. W0 likewise chains
                # behind pwT1 (on gpsimd so it can't block conv1 DVE drains).
                for i in range(2):
                    nc.vector.tensor_copy(pwT[i][0:1, 0:1], P1[0:1, 12799:12800])
                    nc.sync.dma_start(pwT[i][:], pw_t[i * 128:(i + 1) * 128, :])
                for hh in range(2):
                    nc.gpsimd.tensor_copy(W0[hh][0:1, 0:1],
                                          pwT[1][0:1, 20735:20736])
                    nc.sync.dma_start(W0[hh][:], w0_t[hh * 128:(hh + 1) * 128, :])

                for blk in range(2):
                    ps = None
                    for ch in range(25):
                        if ch % 2 == 0:
                            ps = pp1.tile([128, 1024], F32, tag="mm", bufs=3,
                                          name=f"ps{blk}_{ch // 2}")
                        nc.tensor.matmul(ps[:, (ch % 2) * 512:(ch % 2) * 512 + 512],
                                         w1sb[:, blk * 128:(blk + 1) * 128],
                                         P1[:, ch * 512:(ch + 1) * 512],
                                         start=True, stop=True)
                        if ch % 2 == 1 or ch == 24:
                            wid = 1024 if ch % 2 == 1 else 512
                            dst = h_sb[blk][:, (ch // 2) * 1024:(ch // 2) * 1024 + wid]
                            if (ch // 2) % 2 == 0:
                                nc.scalar.activation(dst, ps[:, 0:wid], AF.Relu)
                            else:
                                nc.vector.tensor_scalar_max(dst, ps[:, 0:wid], 0.0)

            # ================= phase 2: primary caps =================
            ph1_scope.__exit__(None, None, None)
            ph2_scope = nc.named_scope("ph2_prim")
            ph2_scope.__enter__()
            with pool("sqz", bufs=2) as sqz, pool("ppB", bufs=8, space="PSUM") as ppB:
                for half in range(2):
                    # psB col layout: (w, b) = ((r2,c2), b); bank j covers r2 in {2j, 2j+1}
                    psB = [ppB.tile([128, 384], F32, tag="psB", name=f"psB{half}_{j}")
                           for j in range(3)]
                    for icblk in range(2):
                        for drdc in range(81):
                            dr, dc = drdc // 9, drdc % 9
                            lhsT = bass.AP(pwT[icblk].tensor,
                                           pwT[icblk].offset + drdc * 256 + half * 128,
                                           [[20736, 128], [1, 128]])
                            for j in range(3):
                                # col = (r2_local, b, c2): c2 innermost for fast streaming
                                rhs = bass.AP(h_sb[icblk].tensor,
                                              h_sb[icblk].offset + dr * 20 + dc + j * 80,
                                              [[12800, 128], [40, 2], [400, 32], [2, 6]])
                                nc.tensor.matmul(psB[j][:], lhsT, rhs,
                                                 start=(icblk == 0 and drdc == 0),
                                                 stop=(icblk == 1 and drdc == 80))
                    # drain: capsP = psB + pb ; sq = (psB + pb)^2 ; reduce to rs
                    sq = sqz.tile([128, 1152], F32, tag="sq", name=f"sq{half}")
                    for j in range(3):
                        nc.scalar.activation(capsP[half][:, j * 384:(j + 1) * 384],
                                             psB[j][:], AF.Identity,
                                             bias=pb_sb[:, half:half + 1])
                        nc.scalar.activation(sq[:, j * 384:(j + 1) * 384],
                                             psB[j][:], AF.Square,
                                             bias=pb_sb[:, half:half + 1])
                    # caps col layout is (r2, b, c2): reduce c2 then r2
                    sqr = sqz.tile([128, 192], F32, tag="sqr", name=f"sqr{half}")
                    nc.vector.tensor_reduce(sqr[:],
                                            sq[:].rearrange("p (rb c) -> p rb c", c=6),
                                            axis=AX.X, op=ALU.add)
                    nc.vector.tensor_reduce(rs[:, half * 32:(half + 1) * 32],
                                            sqr[:].rearrange("p (r b) -> p b r", b=32),
                                            axis=AX.X, op=ALU.add)
            ph2_scope.__exit__(None, None, None)

        pw_pool.__exit__(None, None, None)

        # ================= phase 3: squash + capsT =================
        ph3_scope = nc.named_scope("ph3_squash")
        ph3_scope.__enter__()
        with pool("late") as latep:
            capsT = latep.tile([32, 9216], BF16)
            with pool("pp2", bufs=1, space="PSUM") as pp2:
                psn2 = pp2.tile([8, 32], F32, tag="psn2")
                nc.tensor.matmul(psn2[:], maskL[0][:], rs[:, 0:32], start=True, stop=False)
                nc.tensor.matmul(psn2[:], maskL[1][:], rs[:, 32:64], start=False, stop=True)
                n2sb = constp.tile([8, 32], F32)
                sqrtn = constp.tile([8, 32], F32)
                denom = constp.tile([8, 32], F32)
                rec8 = constp.tile([8, 32], F32)
                sc_lb = constp.tile([8, 32], F32)
                nc.scalar.copy(n2sb[:], psn2[:])
                nc.scalar.sqrt(sqrtn[:], n2sb[:])
                nc.scalar.add(denom[:], n2sb[:], 1.0)
                nc.vector.reciprocal(rec8[:], denom[:])
                nc.vector.tensor_mul(sc_lb[:], sqrtn[:], rec8[:])
                # maskLT[half] = maskL[half].T via PE
                maskLT = []
                for half in range(2):
                    psLT = pp2.tile([8, 128], F32, tag="psLT", bufs=1, name=f"psLT{half}")
                    nc.tensor.transpose(psLT[:], maskL[half][:], identF[:])
                    m = constp.tile([8, 128], F32, name=f"maskLT{half}")
                    nc.vector.tensor_copy(m[:], psLT[:])
                    maskLT.append(m)
                scb_sb = [constp.tile([128, 32], F32, name=f"scb{h}") for h in range(2)]
                for half in range(2):
                    psc_b = pp2.tile([128, 32], F32, tag="pscb", bufs=2, name=f"pscb{half}")
                    nc.tensor.matmul(psc_b[:], maskLT[half][:], sc_lb[:],
                                     start=True, stop=True)
                    nc.vector.tensor_copy(scb_sb[half][:], psc_b[:])
                for half in range(2):
                    nc.vector.tensor_mul(
                        capsS[half][:].rearrange("p (r b c) -> p r b c", b=32, c=6),
                        capsP[half][:].rearrange("p (r b c) -> p r b c", b=32, c=6),
                        bass.AP(scb_sb[half].tensor, scb_sb[half].offset,
                                [[32, 128], [0, 6], [1, 32], [0, 6]]))

                # capsT[b, (w, half, l4, oc)]
                for half in range(2):
                    for g4 in range(9):
                        pst = pp2.tile([32, 512], BF16, tag="pst", bufs=3)
                        for u in range(4):
                            w = g4 * 4 + u
                            src = bass.AP(capsS[half].tensor,
                                          capsS[half].offset + (w // 6) * 192 + (w % 6),
                                          [[1152, 128], [6, 32]])
                            nc.tensor.transpose(pst[:, u * 128:(u + 1) * 128], src, identB[:])
                        dst = bass.AP(capsT.tensor,
                                      capsT.offset + g4 * 4 * 256 + half * 128,
                                      [[9216, 32], [256, 4], [1, 128]])
                        if g4 % 2 == 0:
                            nc.vector.tensor_copy(dst, pst[:])
                        else:
                            nc.scalar.copy(dst, pst[:])
            ph3_scope.__exit__(None, None, None)

            # ================= routing =================
            b_logit = latep.tile([10, 1152], F32)
            nc.gpsimd.memset(b_logit[:], 0.0)
            expb = latep.tile([10, 1152], F32)
            C_sb = latep.tile([10, 1152], BF16)
            C_oc = latep.tile([32, 360], BF16)
            C_arr = latep.tile([128, 360], BF16)
            bupd_red = latep.tile([10, 1152], F32)
            bupd_core = latep.tile([10, 1152], F32)
            Xs = [latep.tile([128, 5760], BF16, name=f"Xs{h}") for h in range(2)]
            Y0 = latep.tile([128, 5760], BF16)
            Y1 = latep.tile([128, 2880], BF16)
            Y2 = latep.tile([128, 1440], BF16)
            Y3 = latep.tile([128, 720], BF16)
            bupd_p = [latep.tile([128, 360], BF16, name=f"bupd_p{h}") for h in range(2)]
            S_sb = constp.tile([32, 160], F32)
            V_sb = constp.tile([32, 160], F32)
            V_bf = constp.tile([32, 160], BF16)
            n2v = constp.tile([32, 16], F32)
            sqv = constp.tile([32, 160], F32)
            sqrtv = constp.tile([32, 16], F32)
            denv = constp.tile([32, 16], F32)
            recv = constp.tile([32, 16], F32)
            scv = constp.tile([32, 16], F32)
            sumexp = constp.tile([10, 1], F32)
            rec10 = constp.tile([10, 1], F32)

            with pool("ardram", bufs=2, space="DRAM") as arp, \
                 pool("ppR", space="PSUM") as ppR:
                for it in range(R_ITERS):
                    sc_S = nc.named_scope(f"rt{it}_S")
                    sc_S.__enter__()
                    # ---------- S matmuls ----------
                    psS = ppR.tile([32, 160], F32, tag="psS", bufs=1, name=f"psS{it}")
                    for half in range(2):
                        if it > 0:
                            Wpc = latep.tile([128, 5760], BF16, tag=f"Wpc{half}",
                                             bufs=1, name=f"Wpc{it}{half}")
                            nc.vector.tensor_mul(
                                Wpc[:].rearrange("p (w k l) -> p w k l", k=16, l=10),
                                W0[half][:].rearrange("p (w k l) -> p w k l", k=16, l=10),
                                C_arr[:].rearrange("p (w l) -> p w l", l=10)
                                    .unsqueeze(2).broadcast_to([128, 36, 16, 10]))
                        for w in range(36):
                            lhsT = bass.AP(capsS[half].tensor,
                                           capsS[half].offset + (w // 6) * 192 + (w % 6),
                                           [[1152, 128], [6, 32]])
                            rhs = (W0[half][:, w * 160:(w + 1) * 160] if it == 0
                                   else Wpc[:, w * 160:(w + 1) * 160])
                            nc.tensor.matmul(psS[:], lhsT, rhs,
                                             start=(half == 0 and w == 0),
                                             stop=(half == 1 and w == 35))
                    # ---------- V = squash_cls(S) ----------
                    nc.scalar.mul(S_sb[:], psS[:], (1.0 / 1152.0) if it == 0 else 1.0)
                    nc.vector.tensor_mul(sqv[:], S_sb[:], S_sb[:])
                    nc.vector.tensor_reduce(n2v[:],
                                            sqv[:].rearrange("p (k l) -> p k l", l=10),
                                            axis=AX.X, op=ALU.add)
                    nc.scalar.sqrt(sqrtv[:], n2v[:])
                    nc.scalar.add(denv[:], n2v[:], 1.0)
                    nc.vector.reciprocal(recv[:], denv[:])
                    nc.vector.tensor_mul(scv[:], sqrtv[:], recv[:])
                    nc.vector.tensor_mul(V_sb[:].rearrange("p (k l) -> p k l", l=10),
                                         S_sb[:].rearrange("p (k l) -> p k l", l=10),
                                         scv[:].unsqueeze(2).broadcast_to([32, 16, 10]))

                    if it == R_ITERS - 1:
                        nc.sync.dma_start(out_t[:, :], V_sb[:])
                        sc_S.__exit__(None, None, None)
                        break
                    nc.scalar.copy(V_bf[:], V_sb[:])
                    sc_S.__exit__(None, None, None)

                    # ---------- X = capsT.T @ V ; Y = X*W0 ; k-tree ----------
                    sc_X = nc.named_scope(f"rt{it}_X")
                    sc_X.__enter__()
                    for half in range(2):
                        for g in range(12):
                            psX = ppR.tile([128, 480], F32, tag="psX", bufs=3,
                                           name=f"psX{it}{half}{g}")
                            for u in range(3):
                                w = g * 3 + u
                                nc.tensor.matmul(
                                    psX[:, u * 160:(u + 1) * 160],
                                    capsT[:, w * 256 + half * 128: w * 256 + half * 128 + 128],
                                    V_bf[:], start=True, stop=True)
                            # drains all on ACT; DVE is the X-phase bottleneck
                            nc.scalar.copy(Xs[half][:, g * 480:(g + 1) * 480], psX[:])
                        nc.vector.tensor_mul(Y0[:], Xs[half][:], W0[half][:])

                        def tap(t, wlen):
                            return bass.AP(t.tensor, t.offset,
                                           [[t.shape[1], 128], [wlen * 10, 36],
                                            [10, wlen], [1, 10]])

                        def tapo(t, wlen, koff):
                            return bass.AP(t.tensor, t.offset + koff * 10,
                                           [[t.shape[1], 128], [wlen * 2 * 10, 36],
                                            [10, wlen], [1, 10]])

                        nc.vector.tensor_add(tap(Y1, 8), tapo(Y0, 8, 0), tapo(Y0, 8, 8))
                        nc.vector.tensor_add(tap(Y2, 4), tapo(Y1, 4, 0), tapo(Y1, 4, 4))
                        nc.vector.tensor_add(tap(Y3, 2), tapo(Y2, 2, 0), tapo(Y2, 2, 2))
                        nc.vector.tensor_add(tap(bupd_p[half], 1),
                                             tapo(Y3, 1, 0), tapo(Y3, 1, 1))
                    # ---------- bupd[10,1152]: sum over m4 + transpose ----------
                    psU = [ppR.tile([10, 384], F32, tag=f"psU{j}", bufs=1,
                                    name=f"psU{it}{j}") for j in range(3)]
                    for half in range(2):
                        for w in range(36):
                            lhsT = bass.AP(bupd_p[half].tensor,
                                           bupd_p[half].offset + w * 10,
                                           [[360, 128], [1, 10]])
                            nc.tensor.matmul(psU[w // 12][:, (w % 12) * 32:(w % 12) * 32 + 32],
                                             lhsT, maskOC[:],
                                             start=(half == 0), stop=(half == 1))
                    sc_X.__exit__(None, None, None)

                    # ---------- AllReduce ----------
                    sc_AR = nc.named_scope(f"rt{it}_AR")
                    sc_AR.__enter__()
                    arin = arp.tile([10, 1152], F32, tag="arin", name=f"arin{it}")
                    arout = arp.tile([10, 1152], F32, tag="arout", addr_space="Shared",
                                     name=f"arout{it}")
                    for j in range(3):
                        dst = bupd_core[:, j * 384:(j + 1) * 384]
                        if j % 2 == 0:
                            nc.vector.tensor_copy(dst, psU[j][:])
                        else:
                            nc.scalar.copy(dst, psU[j][:])
                    nc.sync.dma_start(arin[:], bupd_core[:])
                    if collectives:
                        nc.gpsimd.collective_compute(
                            "AllReduce", ALU.add,
                            replica_groups=[list(range(NC_CORES))],
                            ins=[arin.opt()], outs=[arout.opt()])
                        nc.sync.dma_start(bupd_red[:], arout[:])
                    else:
                        nc.sync.dma_start(bupd_red[:], arin[:])
                    sc_AR.__exit__(None, None, None)

                    # ---------- b += bupd/256 ; C = softmax(b) ----------
                    sc_C = nc.named_scope(f"rt{it}_C")
                    sc_C.__enter__()
                    nc.vector.scalar_tensor_tensor(
                        out=b_logit[:], in0=bupd_red[:], scalar=1.0 / 256.0,
                        in1=b_logit[:], op0=ALU.mult, op1=ALU.add)
                    nc.scalar.activation(expb[:], b_logit[:], AF.Exp,
                                         accum_out=sumexp[:, 0:1])
                    nc.vector.reciprocal(rec10[:], sumexp[:])
                    nc.vector.tensor_scalar_mul(C_sb[:], expb[:], rec10[:, 0:1])
                    # C_oc[oc, (w,l)] via 36 transposes; PE-broadcast to C_arr
                    psC = ppR.tile([32, 360], BF16, tag="psC", bufs=1, name=f"psC{it}")
                    for w in range(36):
                        nc.tensor.transpose(psC[:, w * 10:(w + 1) * 10],
                                            C_sb[:, w * 32:(w + 1) * 32],
                                            identB[0:10, 0:10])
                    nc.vector.tensor_copy(C_oc[:], psC[:])
                    for l4 in range(4):
                        nc.sync.dma_start(C_arr[l4 * 32:(l4 + 1) * 32, :], C_oc[:])
                    sc_C.__exit__(None, None, None)


def build_nc(collectives=True):
    nc = bacc.Bacc("TRN2", target_bir_lowering=False, debug=False,
                   num_devices=NC_CORES)
    io = {
        "p1": nc.dram_tensor("p1", [82, 12800], BF16, kind="ExternalInput").ap(),
        "w1": nc.dram_tensor("w1", [82, 256], BF16, kind="ExternalInput").ap(),
        "pw": nc.dram_tensor("pw", [256, 20736], BF16, kind="ExternalInput").ap(),
        "pb": nc.dram_tensor("pb", [256, 1], F32, kind="ExternalInput").ap(),
        "w0": nc.dram_tensor("w0", [256, 5760], BF16, kind="ExternalInput").ap(),
        "out": nc.dram_tensor("out", [BL, 160], F32, kind="ExternalOutput").ap(),
    }
    with tile.TileContext(nc) as tc:
        build_kernel_body(tc, io, collectives=collectives)
    nc.compile()
    return nc


def make_in_maps(inputs):
    x = np.asarray(inputs["x"], dtype=np.float32).reshape(256, 28, 28)
    w1 = np.asarray(inputs["conv1_w"], dtype=np.float32).reshape(256, 81)
    b1 = np.asarray(inputs["conv1_b"], dtype=np.float32).reshape(256)
    pw = np.asarray(inputs["pri_w"], dtype=np.float32)     # [8,32,256,9,9]
    pb = np.asarray(inputs["pri_b"], dtype=np.float32).reshape(256, 1)
    cw = np.asarray(inputs["caps_W"], dtype=np.float32)    # [1152,8,16,10]

    # w1e [82, 256]: rows 0..80 = w1.T, row 81 = bias
    w1e = np.empty((82, 256), dtype=np.float32)
    w1e[:81] = w1.T
    w1e[81] = b1
    w1e = np.ascontiguousarray(w1e.astype(NPBF))

    # pwT [256 ic, (drdc 81, loc 256)] — stationary [128x128] blocks contiguous
    pwT = np.ascontiguousarray(
        pw.reshape(256, 256, 81).transpose(1, 2, 0).reshape(256, 20736).astype(NPBF))

    # W0A [(m,oc) 256, (w 36, k 16, l 10)]
    w0a = np.ascontiguousarray(
        cw.reshape(32, 36, 8, 16, 10).transpose(2, 0, 1, 3, 4).reshape(256, 5760)
        .astype(NPBF))

    pb_c = np.ascontiguousarray(pb)

    in_maps = []
    for c in range(NC_CORES):
        xc = x[c * BL:(c + 1) * BL]                        # [32,28,28]
        sw = np.lib.stride_tricks.sliding_window_view(xc, (9, 9), axis=(1, 2))
        p1 = np.empty((82, 12800), dtype=np.float32)
        p1[:81] = sw.transpose(3, 4, 0, 1, 2).reshape(81, 12800)
        p1[81] = 1.0
        in_maps.append({
            "p1": np.ascontiguousarray(p1.astype(NPBF)),
            "w1": w1e, "pw": pwT, "pb": pb_c, "w0": w0a,
        })
    return in_maps


_NC_CACHE = None


def kernel(**inputs) -> np.ndarray:
    global _NC_CACHE
    if _NC_CACHE is None:
        _NC_CACHE = build_nc()
    in_maps = make_in_maps(inputs)
    res = run_bass_kernel_spmd(_NC_CACHE, in_maps, core_ids=list(range(NC_CORES)))
    out = np.concatenate([res.results[c]["out"].reshape(BL, 16, 10)
                          for c in range(NC_CORES)], axis=0)
    return out.astype(np.float32)


# revision 54
# speedup vs baseline: 1.3023x; 1.3023x over previous
"""CapsNet Trainium2 kernel — 8-core data-parallel Bass/Tile, bf16 v2.

Per-core (B_loc=32):
  conv1 (9x9,s1): host-side im2col P1[82,12800] (row 81 = ones for bias),
     w1e[82,256] (row 81 = bias) -> 50 matmuls N=512 -> relu -> h bf16
  primary caps (9x9,s2,256ch): host-transposed pwT[ic,(loc,drdc)] bf16,
     162 accumulated matmuls x 4 batch-chunks per half -> psB
     ACT extracts capsP (+bias) and capsP^2 in one pass each
  squash over j, capsT via PE transposes
  routing (3 iters), U never materialized:
     S = capsS @ (W0*C); V = squash_cls(S)
     X = capsT.T @ V; Y = X*W0; k-sum via DVE tree-adds;
     m-sum + transpose fused into per-w mask matmuls -> bupd[10,1152]
     AllReduce -> b += bupd/256 -> C = softmax(b) (PE-broadcast to partitions)
"""
import sys

for _p in ("/opt/trn_rl_repo",):
    if _p not in sys.path:
        sys.path.insert(0, _p)

import numpy as np
import ml_dtypes

import concourse.bass as bass
import concourse.tile as tile
from concourse import bacc, mybir
from concourse.bass_utils import run_bass_kernel_spmd
from concourse.masks import make_identity

F32 = mybir.dt.float32
BF16 = mybir.dt.bfloat16
AF = mybir.ActivationFunctionType
ALU = mybir.AluOpType
AX = mybir.AxisListType

NC_CORES = 8
BL = 32            # batch per core
R_ITERS = 3
NPBF = ml_dtypes.bfloat16


def build_kernel_body(tc, io, collectives=True):
    nc = tc.nc
    p1_t = io["p1"]      # [82, 12800] bf16   im2col(x) + ones row
    w1_t = io["w1"]      # [82, 256]  bf16    conv1 wT + bias row
    pw_t = io["pw"]      # [256, 20736] bf16  [ic, (loc, drdc)]
    pb_t = io["pb"]      # [256, 1]   f32
    w0_t = io["w0"]      # [256, 5760] bf16   [(m,oc), (w, k, l)]
    out_t = io["out"]    # [32, 160]  f32

    def pool(name, bufs=1, space=None):
        kw = {"space": space} if space else {}
        return tc.tile_pool(name=name, bufs=bufs, **kw)

    with pool("const") as constp, pool("caps") as capsp, \
         pool("arwarm", space="DRAM") as arwp:
        # ---------------- early DMAs (big, overlap with conv1) ----------------
        pw_pool = pool("pw")
        pwp = pw_pool.__enter__()
        pwT = [pwp.tile([128, 20736], BF16, name=f"pwT{i}") for i in range(2)]
        W0 = [capsp.tile([128, 5760], BF16, name=f"W0_{h}") for h in range(2)]

        # dummy AllReduce to absorb collective setup latency (runs under conv1)
        if collectives:
            zwarm = constp.tile([10, 16], F32)
            nc.gpsimd.memset(zwarm[:], 0.0)
            arw_in = arwp.tile([10, 16], F32, name="arw_in")
            arw_out = arwp.tile([10, 16], F32, addr_space="Shared", name="arw_out")
            nc.scalar.dma_start(arw_in[:], zwarm[:])
            nc.gpsimd.collective_compute(
                "AllReduce", ALU.add,
                replica_groups=[list(range(NC_CORES))],
                ins=[arw_in.opt()], outs=[arw_out.opt()])

        # ---------------- constants ----------------
        identF = constp.tile([128, 128], F32)
        make_identity(nc, identF[:])
        identB = constp.tile([128, 128], BF16)
        nc.vector.tensor_copy(identB[:], identF[:])

        # maskL[half]: [128,8] f32, 1 at (p, half*4 + p//32)
        maskL = []
        for half in range(2):
            m = constp.tile([128, 8], F32, name=f"maskL{half}")
            nc.gpsimd.memset(m[:], 0.0)
            for l4 in range(4):
                nc.gpsimd.memset(m[l4 * 32:(l4 + 1) * 32, half * 4 + l4: half * 4 + l4 + 1], 1.0)
            maskL.append(m)

        # maskOC: [128,32] 1 at (p, p%32); bf16 copy for bupd matmuls
        mOCa = constp.tile([128, 32], F32)
        mOCb = constp.tile([128, 32], F32)
        maskOCf = constp.tile([128, 32], F32)
        maskOC = constp.tile([128, 32], BF16)
        nc.vector.tensor_add(mOCa[:], identF[:, 0:32], identF[:, 32:64])
        nc.vector.tensor_add(mOCb[:], identF[:, 64:96], identF[:, 96:128])
        nc.vector.tensor_add(maskOCf[:], mOCa[:], mOCb[:])
        nc.vector.tensor_copy(maskOC[:], maskOCf[:])

        pb_sb = constp.tile([128, 2], F32)   # pri bias, col=half
        for blk in range(2):
            nc.sync.dma_start(pb_sb[:, blk:blk + 1], pb_t[blk * 128:(blk + 1) * 128, 0:1])

        capsP = [capsp.tile([128, 1152], F32, name=f"capsP{h}") for h in range(2)]
        capsS = [capsp.tile([128, 1152], BF16, name=f"capsS{h}") for h in range(2)]
        rs = capsp.tile([128, 64], F32)  # per-(l4,oc) sums of squares, col half*32+b

        # ================= phase 1: conv1 =================
        with pool("hbuf") as hp:
            h_sb = [hp.tile([128, 12800], BF16, name=f"h{b}") for b in range(2)]
            ph1_scope = nc.named_scope("ph1_conv1")
            ph1_scope.__enter__()
            with pool("ph1") as ph1, pool("pp1", bufs=8, space="PSUM") as pp1:
                # PE warm-up: dummy matmuls so HAM un-throttles before conv1
                warm = pp1.tile([128, 128], F32, tag="warm", bufs=1)
                for _ in range(24):
                    nc.tensor.matmul(warm[:], identF[:], identF[:, 0:128],
                                     start=True, stop=True)
                w1sb = ph1.tile([82, 256], BF16)
                nc.sync.dma_start(w1sb[:], w1_t[:, :])
                P1 = ph1.tile([82, 12800], BF16)
                for q in range(4):
                    nc.sync.dma_start(P1[:, q * 3200:(q + 1) * 3200],
                                      p1_t[:, q * 3200:(q + 1) * 3200])
                # Big weight loads must not steal HBM bandwidth from the
                # conv1-critical P1: a tiny write into each target tile that
                # reads the LAST P1 chunk creates a WAW dep, so the pwT DMAs
                # start only after P1 has fully landed. W0 likewise chains
                # behind pwT1 (on gpsimd so it can't block conv1 DVE drains).
                for i in range(2):
                    nc.vector.tensor_copy(pwT[i][0:1, 0:1], P1[0:1, 12799:12800])
                    nc.sync.dma_start(pwT[i][:], pw_t[i * 128:(i + 1) * 128, :])
                for hh in range(2):
                    nc.gpsimd.tensor_copy(W0[hh][0:1, 0:1],
                                          pwT[1][0:1, 20735:20736])
                    nc.sync.dma_start(W0[hh][:], w0_t[hh * 128:(hh + 1) * 128, :])

                for blk in range(2):
                    ps = None
                    for ch in range(25):
                        if ch % 2 == 0:
                            ps = pp1.tile([128, 1024], F32, tag="mm", bufs=3,
                                          name=f"ps{blk}_{ch // 2}")
                        nc.tensor.matmul(ps[:, (ch % 2) * 512:(ch % 2) * 512 + 512],
                                         w1sb[:, blk * 128:(blk + 1) * 128],
                                         P1[:, ch * 512:(ch + 1) * 512],
                                         start=True, stop=True)
                        if ch % 2 == 1 or ch == 24:
                            wid = 1024 if ch % 2 == 1 else 512
                            dst = h_sb[blk][:, (ch // 2) * 1024:(ch // 2) * 1024 + wid]
                            if (ch // 2) % 2 == 0:
                                nc.scalar.activation(dst, ps[:, 0:wid], AF.Relu)
                            else:
                                nc.vector.tensor_scalar_max(dst, ps[:, 0:wid], 0.0)

            # ================= phase 2: primary caps =================
            ph1_scope.__exit__(None, None, None)
            ph2_scope = nc.named_scope("ph2_prim")
            ph2_scope.__enter__()
            with pool("sqz", bufs=2) as sqz, pool("ppB", bufs=8, space="PSUM") as ppB:
                for half in range(2):
                    # psB col layout: (w, b) = ((r2,c2), b); bank j covers r2 in {2j, 2j+1}
                    psB = [ppB.tile([128, 384], F32, tag="psB", name=f"psB{half}_{j}")
                           for j in range(3)]
                    for icblk in range(2):
                        for drdc in range(81):
                            dr, dc = drdc // 9, drdc % 9
                            lhsT = bass.AP(pwT[icblk].tensor,
                                           pwT[icblk].offset + half * 128 * 81 + drdc,
                                           [[20736, 128], [81, 128]])
                            for j in range(3):
                                # col = (r2_local, c2, b): contiguous 32-elem b runs
                                rhs = bass.AP(
                                    h_sb[icblk].tensor,
                                    h_sb[icblk].offset + (dc % 2) * 6400 + dr * 320
                                    + (dc // 2) * 32 + j * 1280,
                                    [[12800, 128], [640, 2], [32, 6], [1, 32]])
                                nc.tensor.matmul(psB[j][:], lhsT, rhs,
                                                 start=(icblk == 0 and drdc == 0),
                                                 stop=(icblk == 1 and drdc == 80))
                    # drain: capsP = psB + pb ; sq = (psB + pb)^2 ; reduce to rs
                    sq = sqz.tile([128, 1152], F32, tag="sq", name=f"sq{half}")
                    for j in range(3):
                        nc.scalar.activation(capsP[half][:, j * 384:(j + 1) * 384],
                                             psB[j][:], AF.Identity,
                                             bias=pb_sb[:, half:half + 1])
                        nc.scalar.activation(sq[:, j * 384:(j + 1) * 384],
                                             psB[j][:], AF.Square,
                                             bias=pb_sb[:, half:half + 1])
                    # caps col layout is (w, b): strided reduce over w per b
                    nc.vector.tensor_reduce(rs[:, half * 32:(half + 1) * 32],
                                            sq[:].rearrange("p (w b) -> p b w", b=32),
                                            axis=AX.X, op=ALU.add)
            ph2_scope.__exit__(None, None, None)

        pw_pool.__exit__(None, None, None)

        # ================= phase 3: squash + capsT =================
        ph3_scope = nc.named_scope("ph3_squash")
        ph3_scope.__enter__()
        with pool("late") as latep:
            capsT = latep.tile([32, 9216], BF16)
            with pool("pp2", bufs=1, space="PSUM") as pp2:
                psn2 = pp2.tile([8, 32], F32, tag="psn2")
                nc.tensor.matmul(psn2[:], maskL[0][:], rs[:, 0:32], start=True, stop=False)
                nc.tensor.matmul(psn2[:], maskL[1][:], rs[:, 32:64], start=False, stop=True)
                n2sb = constp.tile([8, 32], F32)
                sqrtn = constp.tile([8, 32], F32)
                denom = constp.tile([8, 32], F32)
                rec8 = constp.tile([8, 32], F32)
                sc_lb = constp.tile([8, 32], F32)
                nc.scalar.copy(n2sb[:], psn2[:])
                nc.scalar.sqrt(sqrtn[:], n2sb[:])
                nc.scalar.add(denom[:], n2sb[:], 1.0)
                nc.vector.reciprocal(rec8[:], denom[:])
                nc.vector.tensor_mul(sc_lb[:], sqrtn[:], rec8[:])
                # maskLT[half] = maskL[half].T via PE
                maskLT = []
                for half in range(2):
                    psLT = pp2.tile([8, 128], F32, tag="psLT", bufs=1, name=f"psLT{half}")
                    nc.tensor.transpose(psLT[:], maskL[half][:], identF[:])
                    m = constp.tile([8, 128], F32, name=f"maskLT{half}")
                    nc.vector.tensor_copy(m[:], psLT[:])
                    maskLT.append(m)
                scb_sb = [constp.tile([128, 32], F32, name=f"scb{h}") for h in range(2)]
                for half in range(2):
                    psc_b = pp2.tile([128, 32], F32, tag="pscb", bufs=2, name=f"pscb{half}")
                    nc.tensor.matmul(psc_b[:], maskLT[half][:], sc_lb[:],
                                     start=True, stop=True)
                    nc.vector.tensor_copy(scb_sb[half][:], psc_b[:])
                for half in range(2):
                    nc.vector.tensor_mul(
                        capsS[half][:].rearrange("p (w b) -> p w b", b=32),
                        capsP[half][:].rearrange("p (w b) -> p w b", b=32),
                        bass.AP(scb_sb[half].tensor, scb_sb[half].offset,
                                [[32, 128], [0, 36], [1, 32]]))

                # capsT[b, (w, half, l4, oc)]
                for half in range(2):
                    for g4 in range(9):
                        pst = pp2.tile([32, 512], BF16, tag="pst", bufs=3)
                        for u in range(4):
                            w = g4 * 4 + u
                            src = bass.AP(capsS[half].tensor,
                                          capsS[half].offset + w * 32,
                                          [[1152, 128], [1, 32]])
                            nc.tensor.transpose(pst[:, u * 128:(u + 1) * 128], src, identB[:])
                        dst = bass.AP(capsT.tensor,
                                      capsT.offset + g4 * 4 * 256 + half * 128,
                                      [[9216, 32], [256, 4], [1, 128]])
                        if g4 % 2 == 0:
                            nc.vector.tensor_copy(dst, pst[:])
                        else:
                            nc.scalar.copy(dst, pst[:])
            ph3_scope.__exit__(None, None, None)

            # ================= routing =================
            b_logit = latep.tile([10, 1152], F32)
            nc.gpsimd.memset(b_logit[:], 0.0)
            expb = latep.tile([10, 1152], BF16)
            C_sb = latep.tile([10, 1152], BF16)
            C_oc = latep.tile([32, 360], BF16)
            C_arr = latep.tile([128, 360], BF16)
            bupd_red = latep.tile([10, 1152], F32)
            bupd_core = latep.tile([10, 1152], F32)
            Xs = [latep.tile([128, 5760], BF16, name=f"Xs{h}") for h in range(2)]
            Y0 = latep.tile([128, 5760], BF16)
            Y1 = latep.tile([128, 2880], BF16)
            Y2 = latep.tile([128, 1440], BF16)
            Y3 = latep.tile([128, 720], BF16)
            bupd_p = [latep.tile([128, 360], BF16, name=f"bupd_p{h}") for h in range(2)]
            S_sb = constp.tile([32, 160], F32)
            V_sb = constp.tile([32, 160], F32)
            V_bf = constp.tile([32, 160], BF16)
            n2v = constp.tile([32, 16], F32)
            sqv = constp.tile([32, 160], F32)
            sqrtv = constp.tile([32, 16], F32)
            denv = constp.tile([32, 16], F32)
            recv = constp.tile([32, 16], F32)
            scv = constp.tile([32, 16], F32)
            sumexp = constp.tile([10, 1], F32)
            rec10 = constp.tile([10, 1], F32)

            with pool("ardram", bufs=2, space="DRAM") as arp, \
                 pool("ppR", space="PSUM") as ppR:
                for it in range(R_ITERS):
                    sc_S = nc.named_scope(f"rt{it}_S")
                    sc_S.__enter__()
                    # ---------- S matmuls ----------
                    psS = ppR.tile([32, 160], F32, tag="psS", bufs=1, name=f"psS{it}")
                    for half in range(2):
                        if it > 0:
                            Wpc = latep.tile([128, 5760], BF16, tag=f"Wpc{half}",
                                             bufs=1, name=f"Wpc{it}{half}")
                            for q in range(6):
                                # chunked so the first S matmuls start early
                                nc.vector.tensor_mul(
                                    Wpc[:, q * 960:(q + 1) * 960]
                                        .rearrange("p (w k l) -> p w k l", k=16, l=10),
                                    W0[half][:, q * 960:(q + 1) * 960]
                                        .rearrange("p (w k l) -> p w k l", k=16, l=10),
                                    C_arr[:, q * 60:(q + 1) * 60]
                                        .rearrange("p (w l) -> p w l", l=10)
                                        .unsqueeze(2).broadcast_to([128, 6, 16, 10]))
                        for w in range(36):
                            lhsT = bass.AP(capsS[half].tensor,
                                           capsS[half].offset + w * 32,
                                           [[1152, 128], [1, 32]])
                            rhs = (W0[half][:, w * 160:(w + 1) * 160] if it == 0
                                   else Wpc[:, w * 160:(w + 1) * 160])
                            nc.tensor.matmul(psS[:], lhsT, rhs,
                                             start=(half == 0 and w == 0),
                                             stop=(half == 1 and w == 35))
                    # ---------- V = squash_cls(S) ----------
                    nc.scalar.mul(S_sb[:], psS[:], (1.0 / 1152.0) if it == 0 else 1.0)
                    nc.vector.tensor_mul(sqv[:], S_sb[:], S_sb[:])
                    nc.vector.tensor_reduce(n2v[:],
                                            sqv[:].rearrange("p (k l) -> p k l", l=10),
                                            axis=AX.X, op=ALU.add)
                    nc.scalar.sqrt(sqrtv[:], n2v[:])
                    nc.scalar.add(denv[:], n2v[:], 1.0)
                    nc.vector.reciprocal(recv[:], denv[:])
                    nc.vector.tensor_mul(scv[:], sqrtv[:], recv[:])
                    nc.vector.tensor_mul(V_sb[:].rearrange("p (k l) -> p k l", l=10),
                                         S_sb[:].rearrange("p (k l) -> p k l", l=10),
                                         scv[:].unsqueeze(2).broadcast_to([32, 16, 10]))

                    if it == R_ITERS - 1:
                        nc.sync.dma_start(out_t[:, :], V_sb[:])
                        sc_S.__exit__(None, None, None)
                        break
                    nc.scalar.copy(V_bf[:], V_sb[:])
                    sc_S.__exit__(None, None, None)

                    # ---------- X = capsT.T @ V ; Y = X*W0 ; k-tree ----------
                    sc_X = nc.named_scope(f"rt{it}_X")
                    sc_X.__enter__()
                    for half in range(2):
                        for g in range(12):
                            psX = ppR.tile([128, 480], F32, tag="psX", bufs=3,
                                           name=f"psX{it}{half}{g}")
                            for u in range(3):
                                w = g * 3 + u
                                nc.tensor.matmul(
                                    psX[:, u * 160:(u + 1) * 160],
                                    capsT[:, w * 256 + half * 128: w * 256 + half * 128 + 128],
                                    V_bf[:], start=True, stop=True)
                            # drains mostly ACT; DVE helps with every third
                            if g % 3 == 2:
                                nc.vector.tensor_copy(
                                    Xs[half][:, g * 480:(g + 1) * 480], psX[:])
                            else:
                                nc.scalar.copy(Xs[half][:, g * 480:(g + 1) * 480],
                                               psX[:])
                        nc.vector.tensor_mul(Y0[:], Xs[half][:], W0[half][:])

                        def tap(t, wlen):
                            return bass.AP(t.tensor, t.offset,
                                           [[t.shape[1], 128], [wlen * 10, 36],
                                            [10, wlen], [1, 10]])

                        def tapo(t, wlen, koff):
                            return bass.AP(t.tensor, t.offset + koff * 10,
                                           [[t.shape[1], 128], [wlen * 2 * 10, 36],
                                            [10, wlen], [1, 10]])

                        nc.vector.tensor_add(tap(Y1, 8), tapo(Y0, 8, 0), tapo(Y0, 8, 8))
                        nc.vector.tensor_add(tap(Y2, 4), tapo(Y1, 4, 0), tapo(Y1, 4, 4))
                        nc.vector.tensor_add(tap(Y3, 2), tapo(Y2, 2, 0), tapo(Y2, 2, 2))
                        nc.vector.tensor_add(tap(bupd_p[half], 1),
                                             tapo(Y3, 1, 0), tapo(Y3, 1, 1))
                    # ---------- bupd[10,1152]: sum over m4 + transpose ----------
                    psU = [ppR.tile([10, 384], F32, tag=f"psU{j}", bufs=1,
                                    name=f"psU{it}{j}") for j in range(3)]
                    for half in range(2):
                        for w in range(36):
                            lhsT = bass.AP(bupd_p[half].tensor,
                                           bupd_p[half].offset + w * 10,
                                           [[360, 128], [1, 10]])
                            nc.tensor.matmul(psU[w // 12][:, (w % 12) * 32:(w % 12) * 32 + 32],
                                             lhsT, maskOC[:],
                                             start=(half == 0), stop=(half == 1))
                    sc_X.__exit__(None, None, None)

                    # ---------- AllReduce ----------
                    sc_AR = nc.named_scope(f"rt{it}_AR")
                    sc_AR.__enter__()
                    arin = arp.tile([10, 1152], F32, tag="arin", name=f"arin{it}")
                    arout = arp.tile([10, 1152], F32, tag="arout", addr_space="Shared",
                                     name=f"arout{it}")
                    for j in range(3):
                        dst = bupd_core[:, j * 384:(j + 1) * 384]
                        if j % 2 == 0:
                            nc.vector.tensor_copy(dst, psU[j][:])
                        else:
                            nc.scalar.copy(dst, psU[j][:])
                    nc.sync.dma_start(arin[:], bupd_core[:])
                    if collectives:
                        nc.gpsimd.collective_compute(
                            "AllReduce", ALU.add,
                            replica_groups=[list(range(NC_CORES))],
                            ins=[arin.opt()], outs=[arout.opt()])
                        nc.sync.dma_start(bupd_red[:], arout[:])
                    else:
                        nc.sync.dma_start(bupd_red[:], arin[:])
                    sc_AR.__exit__(None, None, None)

                    # ---------- b += bupd/256 ; C = softmax(b) ----------
                    sc_C = nc.named_scope(f"rt{it}_C")
                    sc_C.__enter__()
                    nc.vector.scalar_tensor_tensor(
                        out=b_logit[:], in0=bupd_red[:], scalar=1.0 / 256.0,
                        in1=b_logit[:], op0=ALU.mult, op1=ALU.add)
                    nc.scalar.activation(expb[:], b_logit[:], AF.Exp,
                                         accum_out=sumexp[:, 0:1])
                    nc.vector.reciprocal(rec10[:], sumexp[:])
                    nc.vector.tensor_scalar_mul(C_sb[:], expb[:], rec10[:, 0:1])
                    # C_oc[oc, (w,l)] via 36 transposes; PE-broadcast to C_arr
                    psC = ppR.tile([32, 360], BF16, tag="psC", bufs=1, name=f"psC{it}")
                    for w in range(36):
                        nc.tensor.transpose(psC[:, w * 10:(w + 1) * 10],
                                            C_sb[:, w * 32:(w + 1) * 32],
                                            identB[0:10, 0:10])
                    nc.vector.tensor_copy(C_oc[:], psC[:])
                    for l4, eng in enumerate((nc.sync, nc.scalar, nc.gpsimd, nc.sync)):
                        eng.dma_start(C_arr[l4 * 32:(l4 + 1) * 32, :], C_oc[:])
                    sc_C.__exit__(None, None, None)


def build_nc(collectives=True):
    nc = bacc.Bacc("TRN2", target_bir_lowering=False, debug=False,
                   num_devices=NC_CORES)
    io = {
        "p1": nc.dram_tensor("p1", [82, 12800], BF16, kind="ExternalInput").ap(),
        "w1": nc.dram_tensor("w1", [82, 256], BF16, kind="ExternalInput").ap(),
        "pw": nc.dram_tensor("pw", [256, 20736], BF16, kind="ExternalInput").ap(),
        "pb": nc.dram_tensor("pb", [256, 1], F32, kind="ExternalInput").ap(),
        "w0": nc.dram_tensor("w0", [256, 5760], BF16, kind="ExternalInput").ap(),
        "out": nc.dram_tensor("out", [BL, 160], F32, kind="ExternalOutput").ap(),
    }
    with tile.TileContext(nc) as tc:
        build_kernel_body(tc, io, collectives=collectives)
    nc.compile()
    return nc


def make_in_maps(inputs):
    x = np.asarray(inputs["x"], dtype=np.float32).reshape(256, 28, 28)
    w1 = np.asarray(inputs["conv1_w"], dtype=np.float32).reshape(256, 81)
    b1 = np.asarray(inputs["conv1_b"], dtype=np.float32).reshape(256)
    pw = np.asarray(inputs["pri_w"], dtype=np.float32)     # [8,32,256,9,9]
    pb = np.asarray(inputs["pri_b"], dtype=np.float32).reshape(256, 1)
    cw = np.asarray(inputs["caps_W"], dtype=np.float32)    # [1152,8,16,10]

    # w1e [82, 256]: rows 0..80 = w1.T, row 81 = bias
    w1e = np.empty((82, 256), dtype=np.float32)
    w1e[:81] = w1.T
    w1e[81] = b1
    w1e = np.ascontiguousarray(w1e.astype(NPBF))

    # pwT [256 ic, (loc 256, drdc 81)]
    pwT = np.ascontiguousarray(
        pw.reshape(256, 256, 81).transpose(1, 0, 2).reshape(256, 20736).astype(NPBF))

    # W0A [(m,oc) 256, (w 36, k 16, l 10)]
    w0a = np.ascontiguousarray(
        cw.reshape(32, 36, 8, 16, 10).transpose(2, 0, 1, 3, 4).reshape(256, 5760)
        .astype(NPBF))

    pb_c = np.ascontiguousarray(pb)

    in_maps = []
    for c in range(NC_CORES):
        xc = x[c * BL:(c + 1) * BL]                        # [32,28,28]
        sw = np.lib.stride_tricks.sliding_window_view(xc, (9, 9), axis=(1, 2))
        # P1/h column layout: (c-parity 2, r 20, c_half 10, b 32) — b innermost
        # so the stride-2 conv2 rhs streams contiguous 32-element runs
        a = sw.transpose(3, 4, 1, 2, 0)                    # [9,9,20r,20c,32b]
        a = a.reshape(81, 20, 10, 2, 32).transpose(0, 3, 1, 2, 4)
        p1 = np.empty((82, 12800), dtype=np.float32)
        p1[:81] = a.reshape(81, 12800)
        p1[81] = 1.0
        in_maps.append({
            "p1": np.ascontiguousarray(p1.astype(NPBF)),
            "w1": w1e, "pw": pwT, "pb": pb_c, "w0": w0a,
        })
    return in_maps


_NC_CACHE = None


def kernel(**inputs) -> np.ndarray:
    global _NC_CACHE
    if _NC_CACHE is None:
        _NC_CACHE = build_nc()
    in_maps = make_in_maps(inputs)
    res = run_bass_kernel_spmd(_NC_CACHE, in_maps, core_ids=list(range(NC_CORES)))
    out = np.concatenate([res.results[c]["out"].reshape(BL, 16, 10)
                          for c in range(NC_CORES)], axis=0)
    return out.astype(np.float32)
